# revision 1
# baseline (speedup 1.0000x reference)
"""Trainium2 Bass kernel for nn_CLARM_56693568307877.

Computes, for feature sets A [64,640,14,14] and B [128,640,14,14] and a QKV
projection W [240,640]:
    q,k,v = split(x^T W^T); S = q_b k_a^T / sqrt(80); P = softmax(S)
    rec = P v_a;  sim[b,a] = -||v_b - rec||^2_F
Output [128, 64] fp32.

Sharding: data-parallel over the b batch (16 per core x 8 cores);
features_a / W replicated. Everything device-side runs in bf16 with fp32
accumulation (validated: max rel err ~3e-4 vs fp32 reference).

Per-core device pipeline (B=16 b's, A=64 a's, N=M=196 tokens, D=80):
  phase 1: QKV projections on PE (W^T as stationary weights), d-major
           outputs; v_a additionally DMA-transposed to n-major for the
           second attention matmul.
  phase 2: per (a, 4-b subgroup): S^T = k_a^T.T @ q_b^T on PE -> PSUM,
           exp on ACT -> SBUF bf16, U = [v_a|1]^T @ expS^T on PE
           (row 80 = softmax denominator), egress PSUM->SBUF split
           between ACT and DVE.  Denominator rows are DMA-gathered
           across a 4-a wave into a [64,196] tile, inverted with the
           DVE reciprocal, broadcast back via a zero-step DMA, then
           rec = U*w, D = rec - v_b, D^2 and a segmented reduce give
           per-partition sums; a final (-1)-vector matmul reduces over
           partitions into -sum(D^2).
PSUM is managed as a ring of 8 one-bank slots shared by all phases.

Note: this walrus build accepts at most one semaphore wait per
instruction, rejects the custom-DVE ops and InstTensorTensorReduce,
and the xbar DMA-transpose corrupts non-zero-offset destinations;
_split_multi_waits and the scratch-tile transposes work around this.
"""

import numpy as np
import ml_dtypes

import concourse.bass as bass
import concourse.tile as tile
from concourse import mybir
from concourse.bass_utils import run_bass_kernel_spmd

BF16 = mybir.dt.bfloat16
F32 = mybir.dt.float32

NCORES = 8
A_FULL = 64
B_FULL = 128
HID = 640
KC = HID // 128  # 5
N = 196          # tokens (14*14)
D = 80           # inner dim
MPAD = 256       # m padded to 2*128 for clean matmul chunks
SCALE = 1.0 / np.sqrt(D)

_PROGRAM_CACHE = {}


def _build(Asz, Bsz):
    """Emit the Bass program for one core handling Bsz b's x Asz a's."""
    assert Bsz % 4 == 0 and Asz % 4 == 0
    NSG = Bsz // 4            # 4-b subgroups per a
    SG_WAVE = 4 * NSG         # subgroups per 4-a wave
    PW = 4 * Bsz              # pairs per wave

    nc = bass.Bass("TRN2", debug=False)
    fa = nc.dram_tensor("fa", [Asz, KC, 128, N], BF16, kind="ExternalInput")
    fb = nc.dram_tensor("fb", [Bsz, KC, 128, N], BF16, kind="ExternalInput")
    wt = nc.dram_tensor("wt", [KC, 128, 240], BF16, kind="ExternalInput")
    simo = nc.dram_tensor("sim", [Asz, Bsz], F32, kind="ExternalOutput")

    Exp = mybir.ActivationFunctionType.Exp
    mult = mybir.AluOpType.mult
    sub = mybir.AluOpType.subtract
    addop = mybir.AluOpType.add

    with tile.TileContext(nc) as tc:
        with (
            tc.tile_pool(name="const", bufs=1) as cpool,
            tc.tile_pool(name="ring", bufs=1, space="PSUM") as rpool,
            tc.tile_pool(name="x", bufs=3) as x_pool,
            tc.tile_pool(name="vt", bufs=3) as vt_pool,
            tc.tile_pool(name="e", bufs=6) as e_pool,
            tc.tile_pool(name="u", bufs=40) as u_pool,
            tc.tile_pool(name="wbt", bufs=8) as wb_pool,
            tc.tile_pool(name="rec", bufs=4) as rec_pool,
            tc.tile_pool(name="d", bufs=4) as d_pool,
            tc.tile_pool(name="scr", bufs=2) as scr_pool,
            tc.tile_pool(name="stg", bufs=8) as stg_pool,
            tc.tile_pool(name="wave", bufs=2) as wv_pool,
        ):
            wt_sb = cpool.tile([128, KC, 240], BF16, tag="wt")
            kT_all = cpool.tile([128, Asz, MPAD], BF16, tag="kT")
            vaug = cpool.tile([128, Asz, 2, 81], BF16, tag="vaug")
            qT_all = cpool.tile([128, Bsz, N], BF16, tag="qT")
            vbT_all = cpool.tile([80, Bsz, N], BF16, tag="vbT")
            ones_c = cpool.tile([128, 1], F32, tag="ones")
            ones_b = cpool.tile([1, 80], BF16, tag="onesb")
            ring = rpool.tile([128, 8, 512], F32, tag="ring")

            # one-time init
            nc.sync.dma_start(wt_sb, wt.ap().rearrange("k p c -> p k c"))
            nc.gpsimd.memset(kT_all[:], 0.0)
            nc.gpsimd.memset(qT_all[:], 0.0)
            nc.gpsimd.memset(vaug[:], 0.0)
            nc.gpsimd.memset(ones_c[:], 0.0)
            # -1 weights: the final ones-matmul then yields -sum(D^2) directly
            nc.gpsimd.memset(ones_c[0:80, :], -1.0)
            nc.gpsimd.memset(ones_b[:], 1.0)
            nc.gpsimd.memset(vaug[0:128, :, 0, 80:81], 1.0)
            nc.gpsimd.memset(vaug[0:68, :, 1, 80:81], 1.0)

            rp = [0]

            def rslot(k=1):
                s = rp[0] % 8
                rp[0] += k
                return s

            def qkv_batch(src, idx, want):
                """want: 'a' -> (k,v), 'b' -> (q,v); returns dict of psum APs."""
                xt = x_pool.tile([128, KC, N], BF16, tag="x")
                nc.sync.dma_start(xt, src[idx].rearrange("k p n -> p k n"))
                outs = {}
                cols = (("k", 80), ("v", 160)) if want == "a" else (("q", 0), ("v", 160))
                for name, c0 in cols:
                    s = rslot()
                    ps = ring[0:80, s, 0:N]
                    for kc in range(KC):
                        nc.tensor.matmul(
                            ps,
                            wt_sb[:, kc, c0:c0 + 80],
                            xt[:, kc, :],
                            start=(kc == 0),
                            stop=(kc == KC - 1),
                        )
                    outs[name] = ps
                return outs

            # phase 1, b batches
            for b in range(Bsz):
                o = qkv_batch(fb, b, "b")
                nc.scalar.copy(qT_all[0:80, b, :], o["q"])
                nc.scalar.copy(vbT_all[0:80, b, :], o["v"])

            wave_u = []   # (u_sb, sgb) for the current wave
            wave_a0 = 0
            pending = None  # deferred back-half of the previous subgroup

            def _flush_sg(p):
                es, u0, pa, pwidx, psgb, pden = p
                for kc in range(2):
                    for ncx in range(2):
                        nc.tensor.matmul(
                            ring[0:81, u0 + ncx, 0:392],
                            vaug[:, pa, kc, :],
                            es[kc][:, ncx, :],
                            start=(kc == 0),
                            stop=(kc == 1),
                        )
                u_sb = u_pool.tile([81, 2, 392], BF16, tag="u")
                nc.scalar.copy(u_sb[:, 0, :], ring[0:81, u0, 0:392])
                nc.vector.tensor_copy(u_sb[:, 1, :], ring[0:81, u0 + 1, 0:392])
                nc.sync.dma_start(
                    pden[pwidx * 4:(pwidx + 1) * 4, :], u_sb[80:81, :, :]
                )
                wave_u.append((u_sb, psgb))

            for a in range(Asz):
                if a % 4 == 0:
                    wave_a0 = a
                    den_coll = wv_pool.tile([PW, N], BF16, tag="den")
                # phase 1 for this a
                o = qkv_batch(fa, a, "a")
                nc.scalar.copy(kT_all[0:80, a, 0:N], o["k"])
                # vt padded to 256 cols so both DMA transposes are x128 wide;
                # pad cols are zeroed so vaug chunk-1 pad rows stay zero.
                vt = vt_pool.tile([80, MPAD], BF16, tag="vt")
                nc.gpsimd.memset(vt[:, N:MPAD], 0.0)
                nc.scalar.copy(vt[:, 0:N], o["v"])
                # the xbar transpose mangles data at non-zero dst offsets, so
                # transpose into offset-0 scratch tiles and copy into vaug
                t0 = vt_pool.tile([128, 80], BF16, tag="vtr0")
                t1 = vt_pool.tile([128, 80], BF16, tag="vtr1")
                nc.sync.dma_start_transpose(t0, vt[:, 0:128])
                nc.sync.dma_start_transpose(t1, vt[:, 128:MPAD])
                nc.vector.tensor_copy(vaug[0:128, a, 0, 0:80], t0)
                nc.vector.tensor_copy(vaug[0:68, a, 1, 0:80], t1[0:68, :])

                for sgb in range(NSG):
                    b0 = 4 * sgb
                    widx = (a % 4) * NSG + sgb  # subgroup index in wave
                    # front half: mm1 + exp; ring slots for U reserved now
                    sbank = []
                    for mc in range(2):
                        s0 = rslot(2)
                        assert s0 % 2 == 0
                        sbank.append(s0)
                        for ncx in range(2):
                            nc.tensor.matmul(
                                ring[:, s0 + ncx, 0:392],
                                kT_all[:, a, mc * 128:(mc + 1) * 128],
                                qT_all[:, b0 + 2 * ncx: b0 + 2 * ncx + 2, :],
                                start=True,
                                stop=True,
                            )
                    es = []
                    for mc in range(2):
                        e = e_pool.tile([128, 2, 392], BF16, tag="e")
                        nc.scalar.activation(
                            e, ring[:, sbank[mc]:sbank[mc] + 2, 0:392], Exp
                        )
                        es.append(e)
                    u0 = rslot(2)
                    assert u0 % 2 == 0
                    # back half of the PREVIOUS subgroup is emitted here so
                    # the PE can run this sg's mm1 while exp(prev) finishes
                    if pending is not None:
                        _flush_sg(pending)
                    pending = (es, u0, a, widx, sgb, den_coll)

                if a % 4 == 3:
                    if pending is not None:
                        _flush_sg(pending)
                        pending = None
                    # wave tail: reciprocal + broadcast + rec/D/reduce
                    den_f = wv_pool.tile([PW, N], F32, tag="denf")
                    nc.vector.tensor_copy(den_f, den_coll)
                    w_f = wv_pool.tile([PW, N], F32, tag="wf")
                    nc.vector.reciprocal(w_f, den_f)
                    w_b = wv_pool.tile([PW, N], BF16, tag="wb")
                    nc.vector.tensor_copy(w_b, w_f)
                    simcol = wv_pool.tile([128, PW], F32, tag="sc")
                    nc.vector.memset(simcol[:], 0.0)
                    for wi, (u_sb, sgb) in enumerate(wave_u):
                        wb_t = wb_pool.tile([80, 4, N], BF16, tag="wbt")
                        stg = stg_pool.tile([1, 4, N], BF16, tag="stg")
                        nc.sync.dma_start(stg, w_b[wi * 4:(wi + 1) * 4, :])
                        src_b = bass.AP(
                            stg.tensor, stg.offset, [[1, 1], [0, 80], [1, 4 * N]]
                        )
                        nc.sync.dma_start(wb_t, src_b)
                        rec = rec_pool.tile([80, 2, 392], BF16, tag="rec")
                        nc.vector.tensor_tensor(rec, u_sb[0:80, :, :], wb_t, op=mult)
                        d_t = d_pool.tile([80, 2, 392], BF16, tag="d")
                        nc.vector.tensor_tensor(
                            d_t, rec, vbT_all[:, 4 * sgb:4 * sgb + 4, :], op=sub
                        )
                        d2 = scr_pool.tile([80, 4, N], BF16, tag="scr")
                        dv = d_t.rearrange("p c x -> p (c x)").rearrange(
                            "p (j n) -> p j n", j=4
                        )
                        nc.vector.tensor_tensor(d2, dv, dv, op=mult)
                        nc.vector.reduce_sum(
                            out=simcol[0:80, wi * 4:(wi + 1) * 4],
                            in_=d2,
                            axis=mybir.AxisListType.X,
                        )
                    wave_u = []
                    # reduce over the 80 d-partitions with a ones matmul
                    cs = rslot(2)
                    nc.tensor.matmul(
                        ring[0:1, cs, 0:PW], ones_c[:, 0:1], simcol[:],
                        start=True, stop=True,
                    )
                    simrow = wv_pool.tile([1, PW], F32, tag="sr")
                    nc.vector.tensor_copy(simrow, ring[0:1, cs, 0:PW])
                    # simrow layout [a_local, b] matches simo rows wave_a0..+4
                    nc.sync.dma_start(simo[wave_a0:wave_a0 + 4, :], simrow[0:1, :])

    return nc


def _split_multi_waits(nc):
    """This walrus build accepts at most one semaphore wait per instruction;
    Tile emits several (incl. its tail drain). Hoist extra waits onto
    single-wait engine NoOps inserted just before the instruction."""
    cnt = 0
    for f in nc.m.functions:
        for bb in f.blocks:
            insts = list(bb.instructions)
            out = []
            changed = False
            for inst in insts:
                si = getattr(inst, "sync_info", None)
                ws = list(si.on_wait) if (si is not None and si.on_wait) else []
                if len(ws) > 1:
                    changed = True
                    for w in ws[:-1]:
                        cnt += 1
                        out.append(mybir.InstNoOp(
                            name=f"WSPLIT-{cnt}",
                            engine=inst.engine,
                            ins=[], outs=[],
                            sync_info=mybir.SyncInfo(on_wait=[w], on_update=[]),
                        ))
                    si.on_wait = [ws[-1]]
                    inst.sync_info = si
                out.append(inst)
            if changed:
                bb.instructions = out
    return nc


def _get_program(Asz, Bsz):
    key = (Asz, Bsz)
    if key not in _PROGRAM_CACHE:
        _PROGRAM_CACHE[key] = _split_multi_waits(_build(Asz, Bsz))
    return _PROGRAM_CACHE[key]


def _prep_inputs(features_a, features_b, W_qkv, Asz, Bsz, ncores):
    """Host-side: cast to bf16, fold the 1/sqrt(D) scale into Wq, reshape."""
    fa = features_a.reshape(Asz, HID, N).astype(ml_dtypes.bfloat16)
    fa = fa.reshape(Asz, KC, 128, N)
    wt = W_qkv.T.copy().astype(np.float32)   # [640, 240]
    wt[:, 0:D] *= SCALE
    wt = wt.astype(ml_dtypes.bfloat16).reshape(KC, 128, 240)
    fbs = []
    for c in range(ncores):
        fb = features_b[c * Bsz:(c + 1) * Bsz].reshape(Bsz, HID, N)
        fb = fb.astype(ml_dtypes.bfloat16).reshape(Bsz, KC, 128, N)
        fbs.append(fb)
    return fa, fbs, wt


def kernel(features_a, features_b, W_qkv):
    Asz = features_a.shape[0]
    Bfull = features_b.shape[0]
    ncores = NCORES
    Bsz = Bfull // ncores
    fa, fbs, wt = _prep_inputs(
        np.asarray(features_a), np.asarray(features_b), np.asarray(W_qkv),
        Asz, Bsz, ncores,
    )
    nc = _get_program(Asz, Bsz)
    in_maps = [{"fa": fa, "fb": fbs[c], "wt": wt} for c in range(ncores)]
    res = run_bass_kernel_spmd(nc, in_maps, core_ids=list(range(ncores)))
    out = np.concatenate([res.results[c]["sim"].T for c in range(ncores)], axis=0)
    return out.astype(np.float32)



# revision 4
# speedup vs baseline: 1.1167x; 1.1167x over previous
"""Trainium2 Bass kernel for nn_CLARM_56693568307877 — v2 (restructured).

Math: q,k,v = x^T W^T per batch; S = q_b k_a^T / sqrt(80); P = softmax(S);
rec = P v_a; sim[b,a] = -||v_b - rec||^2.

v2 reformulation (per (b,a) pair, with U = E v_a, E = exp(S - C), den = sum_m E):
  sim = -K_b + sum_n [ 2 w c1 - w^2 c2 ],  w = 1/den
  c1[n] = sum_d v_b[d,n] U[d,n],  c2[n] = sum_d U[d,n]^2,  K_b = ||v_b||_F^2
The constant shift C in exp cancels exactly and keeps E inside fp8e4 range.

Device mapping (per core: A=64 a's, B=16 b's, data-parallel over b across
8 cores):
- QKV on PE as fp8 DoubleRow matmuls (W^T stationary, x/W in fp8);
  q/k cast to fp8 and repacked to [40,2,*] (d split in two K-subtiles) by
  SP-issued DMAs; v_a transposed via PE-transpose matmuls and cast to fp8
  [128,2,80] (m split 128+68+pad).
- mm1 (S^T) and mm2 (U) and the den-reduction run as fp8 DoubleRow
  matmuls (0.5 cycles/col).
- c1/c2/den are reduced over partitions by ones-column matmuls whose
  stationary strip places subgroup j's result on its own row of a shared
  per-wave PSUM accumulator bank (6 streams x 16 rows); zero gather or
  broadcast DMAs remain.
- Wave tail: reciprocal + a few [32,392] DVE ops + reduce + one DMA out.

Subgroups are emitted as software-pipelined pairs (even/odd mm1 bank
parity) so every exp's PE feeder runs under the previous exp.
PSUM banks: 0,1 mm1 even-sg (chunk-serial); 2,3 mm1 odd-sg; 4,5 U(ncx0/1)
+ bf16 transpose scratch in bank-4 tail; 6,7 wave accumulator (ping-pong;
the idle one hosts the next wave's QKV psum).
"""

import numpy as np
import ml_dtypes

import concourse.bass as bass
import concourse.tile as tile
from concourse import mybir
from concourse.bass_utils import run_bass_kernel_spmd

BF16 = mybir.dt.bfloat16
F32 = mybir.dt.float32
F8 = mybir.dt.float8e4

NCORES = 8
A_FULL = 64
B_FULL = 128
HID = 640
KC = HID // 128          # 5
N = 196
D = 80
SCALE = 1.0 / np.sqrt(D)
CSH = 1.5                # exp shift; cancels in softmax, keeps E in fp8 range
DR = mybir.MatmulPerfMode.DoubleRow

_PROGRAM_CACHE = {}


def _build(Asz, Bsz):
    assert Asz % 4 == 0 and Bsz == 16
    NW = Asz // 4            # waves (4 a's x 4 subgroups of 4 b's = 16 sg)

    nc = bass.Bass("TRN2", debug=False)
    fa = nc.dram_tensor("fa", [Asz, 128, KC, N], BF16, kind="ExternalInput")
    fb = nc.dram_tensor("fb", [Bsz, 128, KC, N], BF16, kind="ExternalInput")
    wt = nc.dram_tensor("wt", [128, KC, 240], BF16, kind="ExternalInput")
    ident = nc.dram_tensor("ident", [80, 80], BF16, kind="ExternalInput")
    cst2 = nc.dram_tensor("cst2", [80, 384], BF16, kind="ExternalInput")
    cst1 = nc.dram_tensor("cst1", [80, 384], BF16, kind="ExternalInput")
    dst8 = nc.dram_tensor("dst8", [128, 2, 384], F8, kind="ExternalInput")
    selk = nc.dram_tensor("selk", [8, 32], F32, kind="ExternalInput")
    simo = nc.dram_tensor("sim", [Asz, Bsz], F32, kind="ExternalOutput")

    Exp = mybir.ActivationFunctionType.Exp
    Square = mybir.ActivationFunctionType.Square
    mult = mybir.AluOpType.mult
    sub = mybir.AluOpType.subtract

    with tile.TileContext(nc) as tc:
        with (
            tc.tile_pool(name="const", bufs=1) as cpool,
            tc.tile_pool(name="ring", bufs=1, space="PSUM") as rpool,
            tc.tile_pool(name="x", bufs=5) as x_pool,
            tc.tile_pool(name="q8", bufs=4) as q8_pool,
            tc.tile_pool(name="vt", bufs=3) as vt_pool,
            tc.tile_pool(name="sq", bufs=2) as sq_pool,
            tc.tile_pool(name="es", bufs=3) as es_pool,
            tc.tile_pool(name="tt", bufs=6) as tt_pool,
            tc.tile_pool(name="u32", bufs=3) as u32_pool,
            tc.tile_pool(name="wv", bufs=10) as wv_pool,
            tc.tile_pool(name="zz", bufs=4) as zz_pool,
        ):
            wt_sb = cpool.tile([128, KC, 240], BF16, tag="wt")
            ident_sb = cpool.tile([80, 80], BF16, tag="ident")
            cst2_sb = cpool.tile([80, 384], BF16, tag="cst2")
            cst1_sb = cpool.tile([80, 384], BF16, tag="cst1")
            dst8_sb = cpool.tile([128, 2, 384], F8, tag="dst8")
            qT8 = cpool.tile([40, 2, Bsz, N], F8, tag="qT8")
            kT8 = cpool.tile([40, 2, Asz, 256], F8, tag="kT8")
            va8 = cpool.tile([128, 2, Asz, 80], F8, tag="va8")
            vbT = cpool.tile([80, Bsz, N], BF16, tag="vbT")
            Ksq = cpool.tile([80, Bsz], F32, tag="Ksq")
            Krow = cpool.tile([1, Bsz], F32, tag="Krow")
            K8x2 = cpool.tile([8, 2], F32, tag="K8x2")
            selk_sb = cpool.tile([8, 32], F32, tag="selk_sb")
            K32v = cpool.tile([32, 2], F32, tag="K32v")
            ones80 = cpool.tile([80, 1], F32, tag="ones80")
            biasc = cpool.tile([128, 1], F32, tag="biasc")
            ring = rpool.tile([128, 8, 512], F32, tag="ring")

            # --- init ---
            nc.sync.dma_start(wt_sb, wt.ap())
            nc.sync.dma_start(ident_sb, ident.ap())
            nc.sync.dma_start(cst2_sb, cst2.ap())
            nc.sync.dma_start(cst1_sb, cst1.ap())
            nc.sync.dma_start(dst8_sb, dst8.ap())
            nc.sync.dma_start(selk_sb, selk.ap())
            nc.vector.memset(kT8[:, :, :, N:256], 0.0)
            nc.vector.memset(ones80[:], 1.0)
            nc.vector.memset(biasc[:], -CSH)

            # psum views: banks 0,1 mm1-even; 2,3 mm1-odd; 4,5 U (+tp in
            # bank4 tail); 6,7 wave accumulators (ping-pong, idle one hosts
            # the next wave's QKV psum).
            def mm1b(j):
                b0 = 2 * (j % 2)
                return (ring[:, b0, 0:392], ring[:, b0 + 1, 0:392],
                        ring[:, b0:b0 + 2, 0:392])

            Cu = ring[0:80, 4, 0:392]
            Du = ring[0:80, 5, 0:392]
            Uboth = ring[0:80, 4:6, 0:392]
            kwps = ring[0:1, 6, 400:416]
            k32ps = ring[0:32, 6, 392:394]
            def tp_views(bank):
                tpx = ring[:, bank, 416:496].bitcast(BF16)
                tpr = tpx.rearrange("p (t d) -> p t d", t=2)
                return tpx, tpr[:, 0, :], tpr[:, 1, :]

            def qkv_mm(xt, c0, out_ps):
                for kc in range(KC):
                    nc.tensor.matmul(
                        out_ps, wt_sb[:, kc, c0:c0 + 80], xt[:, kc, :],
                        start=(kc == 0), stop=(kc == KC - 1),
                    )

            # --- phase 1: b batches (bank 6, before wave 0) ---
            for b in range(Bsz):
                bank = 6 + (b % 2)
                xt = x_pool.tile([128, KC, N], BF16, tag="x")
                nc.sync.dma_start(xt, fb[b])
                qps = ring[0:80, bank, 0:N]
                vps = ring[0:80, bank, 200:200 + N]
                qkv_mm(xt, 0, qps)
                qkv_mm(xt, 160, vps)
                q8t = q8_pool.tile([80, N], F8, tag="q8")
                nc.vector.tensor_copy(q8t, qps)
                nc.sync.dma_start(qT8[:, 0, b, :], q8t[0:40, :])
                nc.sync.dma_start(qT8[:, 1, b, :], q8t[40:80, :])
                nc.vector.tensor_copy(vbT[:, b, :], vps)
                sqt = sq_pool.tile([80, N], BF16, tag="sq")
                nc.scalar.activation(
                    sqt, vps, Square, accum_out=Ksq[:, b:b + 1]
                )

            # K_b finalize: Krow = ones80^T @ Ksq (f32 matmul), then
            # K8x2[p, bi] = Krow[2p + bi]; K32v = selk^T @ K8x2 with
            # selk[p, j'] = (p == 2*(j'%4) + j'//16)
            nc.tensor.matmul(kwps, ones80, Ksq, start=True, stop=True)
            nc.vector.tensor_copy(Krow, kwps)
            src = bass.AP(Krow.tensor, Krow.offset, [[1, 1], [2, 8], [1, 2]])
            nc.sync.dma_start(K8x2, src)
            nc.tensor.matmul(k32ps, selk_sb, K8x2, start=True, stop=True)
            nc.vector.tensor_copy(K32v, k32ps)

            # --- a-batch QKV into the idle wave bank ---
            def qkv_a(a, bank):
                xt = x_pool.tile([128, KC, N], BF16, tag="x")
                nc.sync.dma_start(xt, fa[a])
                qps = ring[0:80, bank, 0:N]
                vps = ring[0:80, bank, 200:200 + N]
                qkv_mm(xt, 80, qps)
                qkv_mm(xt, 160, vps)
                k8t = q8_pool.tile([80, N], F8, tag="q8")
                nc.scalar.copy(k8t, qps)
                nc.sync.dma_start(kT8[:, 0, a, 0:N], k8t[0:40, :])
                nc.sync.dma_start(kT8[:, 1, a, 0:N], k8t[40:80, :])
                vtt = vt_pool.tile([80, 256], BF16, tag="vt")
                nc.gpsimd.memset(vtt[:, N:256], 0.0)
                nc.scalar.copy(vtt[:, 0:N], vps)
                tpx, tp0, tp1 = tp_views(bank)
                nc.tensor.transpose(tp0, vtt[:, 0:128], ident_sb)
                nc.tensor.transpose(tp1, vtt[:, 128:256], ident_sb)
                nc.vector.tensor_copy(va8[:, :, a, :], tpx)

            for a in range(4):
                qkv_a(a, 7)

            # --- attention waves, software-pipelined emission ---
            def P1(w, j, wb):
                a = 4 * w + j // 4
                b0 = 4 * (j % 4)
                A, B, _ = mm1b(j)
                nc.tensor.matmul(A, kT8[:, :, a, 0:128],
                                 qT8[:, :, b0:b0 + 2, :],
                                 start=True, stop=True, perf_mode=DR)
                nc.tensor.matmul(B, kT8[:, :, a, 0:128],
                                 qT8[:, :, b0 + 2:b0 + 4, :],
                                 start=True, stop=True, perf_mode=DR)

            def P2(w, j):
                a = 4 * w + j // 4
                b0 = 4 * (j % 4)
                A, B, _ = mm1b(j)
                nc.tensor.matmul(A, kT8[:, :, a, 128:256],
                                 qT8[:, :, b0:b0 + 2, :],
                                 start=True, stop=True, perf_mode=DR)
                nc.tensor.matmul(B, kT8[:, :, a, 128:256],
                                 qT8[:, :, b0 + 2:b0 + 4, :],
                                 start=True, stop=True, perf_mode=DR)

            def E(j, es, kt):
                _, _, expin = mm1b(j)
                nc.scalar.activation(es[:, kt, :, :], expin, Exp,
                                     bias=biasc[:])

            def P3(w, j, es, wb):
                a = 4 * w + j // 4
                nc.tensor.matmul(Cu, va8[:, :, a, :], es[:, :, 0, :],
                                 start=True, stop=True, perf_mode=DR)
                nc.tensor.matmul(Du, va8[:, :, a, :], es[:, :, 1, :],
                                 start=True, stop=True, perf_mode=DR)
                nc.tensor.matmul(wb, dst8_sb[:, :, 192 - j:320 - j],
                                 es[:, :, 0, :],
                                 start=(j == 0), stop=False, perf_mode=DR)
                nc.tensor.matmul(wb, dst8_sb[:, :, 176 - j:304 - j],
                                 es[:, :, 1, :],
                                 start=False, stop=False, perf_mode=DR)

            def V(j):
                b0 = 4 * (j % 4)
                t1 = tt_pool.tile([80, 2, 392], BF16, tag="tt")
                t2 = tt_pool.tile([80, 2, 392], BF16, tag="tt")
                u16 = u32_pool.tile([80, 2, 392], BF16, tag="u16")
                nc.vector.tensor_copy(u16, Uboth)
                nc.vector.tensor_tensor(t1, u16, vbT[:, b0:b0 + 4, :],
                                        op=mult)
                nc.vector.tensor_tensor(t2[:, 0, 0:150], u16[:, 0, 0:150],
                                        u16[:, 0, 0:150], op=mult)
                nc.gpsimd.tensor_tensor(t2[:, 0, 150:392], u16[:, 0, 150:392],
                                        u16[:, 0, 150:392], op=mult)
                nc.gpsimd.tensor_tensor(t2[:, 1, :], u16[:, 1, :],
                                        u16[:, 1, :], op=mult)
                return (t1, t2)

            def P4(t1t2, j, wb, last):
                t1, t2 = t1t2
                nc.tensor.matmul(wb, cst2_sb[:, 256 - j:384 - j], t1[:, 0, :],
                                 start=False, stop=False)
                nc.tensor.matmul(wb, cst2_sb[:, 240 - j:368 - j], t1[:, 1, :],
                                 start=False, stop=False)
                nc.tensor.matmul(wb, cst1_sb[:, 224 - j:352 - j], t2[:, 0, :],
                                 start=False, stop=False)
                nc.tensor.matmul(wb, cst1_sb[:, 208 - j:336 - j], t2[:, 1, :],
                                 start=False, stop=last)

            def tail(w):
                wbank = 6 + (w % 2)
                w32 = wv_pool.tile([32, 392], F32, tag="wv")
                e1 = wv_pool.tile([32, 392], F32, tag="wv")
                e2 = wv_pool.tile([32, 392], F32, tag="wv")
                e2p = wv_pool.tile([32, 392], F32, tag="wv")
                zt = wv_pool.tile([32, 392], F32, tag="wv")
                nc.vector.reciprocal(w32, ring[64:96, wbank, 0:392])
                nc.vector.tensor_tensor(e1, w32, ring[0:32, wbank, 0:392],
                                        op=mult)
                nc.vector.tensor_tensor(e2, w32, ring[32:64, wbank, 0:392],
                                        op=mult)
                nc.gpsimd.tensor_tensor(e2p, w32, e2, op=mult)
                nc.gpsimd.tensor_tensor(zt, e1, e2p, op=sub)
                zr = zz_pool.tile([32, 2], F32, tag="zr")
                nc.vector.reduce_sum(
                    out=zr, in_=zt.rearrange("p (c n) -> p c n", c=2),
                    axis=mybir.AxisListType.X,
                )
                simf = zz_pool.tile([32, 2], F32, tag="simf")
                nc.vector.tensor_tensor(simf, zr, K32v, op=sub)
                for h in range(2):
                    dst = bass.AP(
                        simo.ap().tensor, 4 * w * Bsz + 2 * h,
                        [[Bsz, 4], [4, 4], [1, 2]],
                    )
                    nc.sync.dma_start(dst, simf[16 * h:16 * h + 16, :])

            NSG = 16 * NW
            qkv_pts = {4: 0, 7: 1, 10: 2, 13: 3}

            def do_p4(g):
                pw, pj = g // 16, g % 16
                P4(vq[g], pj, ring[:, 6 + (pw % 2), 0:392], pj == 15)
                if pj == 15:
                    tail(pw)

            vq = {}
            es_cur = es_pool.tile([128, 2, 2, 392], F8, tag="es")
            P1(0, 0, None)
            E(0, es_cur, 0)
            for g in range(NSG):
                w, j = g // 16, g % 16
                wb = ring[:, 6 + (w % 2), 0:392]
                P2(w, j)
                E(j, es_cur, 1)
                es_nxt = None
                if g + 1 < NSG:
                    es_nxt = es_pool.tile([128, 2, 2, 392], F8, tag="es")
                    P1((g + 1) // 16, (g + 1) % 16, None)
                    E((g + 1) % 16, es_nxt, 0)
                if g - 2 >= 0:
                    do_p4(g - 2)
                    del vq[g - 2]
                if j in qkv_pts and w + 1 < NW:
                    qkv_a(4 * (w + 1) + qkv_pts[j], 6 + ((w + 1) % 2))
                P3(w, j, es_cur, wb)
                vq[g] = V(j)
                es_cur = es_nxt
            do_p4(NSG - 2)
            do_p4(NSG - 1)

    return nc


def _split_multi_waits(nc):
    """This walrus build accepts at most one semaphore wait per instruction;
    hoist extra waits onto single-wait engine NoOps."""
    cnt = 0
    for f in nc.m.functions:
        for bb in f.blocks:
            insts = list(bb.instructions)
            out = []
            changed = False
            for inst in insts:
                si = getattr(inst, "sync_info", None)
                ws = list(si.on_wait) if (si is not None and si.on_wait) else []
                if len(ws) > 1:
                    changed = True
                    for wv in ws[:-1]:
                        cnt += 1
                        out.append(mybir.InstNoOp(
                            name=f"WSPLIT-{cnt}",
                            engine=inst.engine,
                            ins=[], outs=[],
                            sync_info=mybir.SyncInfo(on_wait=[wv], on_update=[]),
                        ))
                    si.on_wait = [ws[-1]]
                    inst.sync_info = si
                out.append(inst)
            if changed:
                bb.instructions = out
    return nc


def _get_program(Asz, Bsz):
    key = (Asz, Bsz)
    if key not in _PROGRAM_CACHE:
        _PROGRAM_CACHE[key] = _split_multi_waits(_build(Asz, Bsz))
    return _PROGRAM_CACHE[key]


def _consts():
    f8 = ml_dtypes.float8_e4m3
    bf = ml_dtypes.bfloat16
    ident = np.eye(80, dtype=np.float32).astype(bf)
    cst2 = np.zeros((80, 384), np.float32)
    cst2[:, 256] = 2.0
    cst1 = np.zeros((80, 384), np.float32)
    cst1[:, 256] = 1.0
    dst8 = np.zeros((128, 2, 384), np.float32)
    dst8[:, 0, 256] = 1.0
    dst8[0:68, 1, 256] = 1.0
    selk = np.zeros((8, 32), np.float32)
    for j in range(32):
        selk[2 * (j % 4) + j // 16, j] = 1.0
    return {
        "ident": ident,
        "cst2": cst2.astype(bf),
        "cst1": cst1.astype(bf),
        "dst8": dst8.astype(f8),
        "selk": selk,
    }


def _prep_inputs(features_a, features_b, W_qkv, Asz, Bsz, ncores):
    bf = ml_dtypes.bfloat16
    # [a, hid, n] -> [a, 128, kc, n] partition-major for fat DMA descriptors
    fa = features_a.reshape(Asz, KC, 128, N).transpose(0, 2, 1, 3)
    fa = np.ascontiguousarray(fa).astype(bf)
    wt = W_qkv.T.copy().astype(np.float32)       # [640, 240]
    wt[:, 0:D] *= SCALE
    wt = wt.reshape(KC, 128, 240).transpose(1, 0, 2)
    wt = np.ascontiguousarray(wt).astype(bf)
    fbs = []
    for c in range(ncores):
        fb = features_b[c * Bsz:(c + 1) * Bsz].reshape(Bsz, KC, 128, N)
        fb = np.ascontiguousarray(fb.transpose(0, 2, 1, 3)).astype(bf)
        fbs.append(fb)
    return fa, fbs, wt


def kernel(features_a, features_b, W_qkv):
    Asz = features_a.shape[0]
    Bfull = features_b.shape[0]
    ncores = NCORES
    Bsz = Bfull // ncores
    features_a = np.asarray(features_a).reshape(Asz, HID, N)
    features_b = np.asarray(features_b).reshape(Bfull, HID, N)
    fa, fbs, wt = _prep_inputs(
        features_a, features_b, np.asarray(W_qkv), Asz, Bsz, ncores)
    consts = _consts()
    nc = _get_program(Asz, Bsz)
    in_maps = [dict(fa=fa, fb=fbs[c], wt=wt, **consts) for c in range(ncores)]
    res = run_bass_kernel_spmd(nc, in_maps, core_ids=list(range(ncores)))
    out = np.concatenate(
        [res.results[c]["sim"].T for c in range(ncores)], axis=0)
    return out.astype(np.float32)
